# revision 27
# baseline (speedup 1.0000x reference)
"""C51 categorical-support projection kernel for Trainium2 (8 NeuronCores).

Algorithm: target[i,k] = sum_j p[i,j] * relu(1 - |b[i,j] - k|) where
b[i,j] = clip(r_i + gamma*(1-done_i)*z_j + 300, 0, 600).  For non-done rows
b[i,j] - j = r_i + (1-gamma)*(300-j) stays within a small window, so the
scatter is computed as a sum over ~17 diagonal shifts d with per-shift
trimmed j-windows.  Done rows (beta=0) are handled by initializing the
accumulator with the one-hot pair relu(1-|b_i - k|) (their p-mass is zeroed
via a premultiplied -(1-done) factor, and sum_j p = 1 for softmax rows).

Data-parallel over the batch across 8 cores; no cross-core comms.
"""

import numpy as np

import concourse.bacc as bacc
import concourse.bass as bass
import concourse.mybir as mybir
from concourse.bass_utils import run_bass_kernel_spmd
from concourse.tile import TileContext

f32 = mybir.dt.float32
Alu = mybir.AluOpType
Act = mybir.ActivationFunctionType

NCORES = 8
B_FULL = 131072
N = 601
PB = B_FULL // NCORES  # 16384 rows per core
P = 128                # SBUF partitions
T = 4                  # row-tiles fused per instruction
NOUTER = PB // (P * T) # 32 outer iterations

# engine cost model, ns per element per partition-lane
_COST = {
    ("vector", "ts"): 0.52,   # DVE 2x mode (1 tensor read)
    ("vector", "tt"): 1.042,  # DVE 1x (2 tensor reads)
    ("scalar", "act"): 0.833, # ACT @1.2GHz
    ("gpsimd", "ts"): 0.833,  # POOL 1-input line rate
    ("gpsimd", "tt"): 1.667,  # POOL 2-input
}
_INSTR_OVERHEAD = 80.0  # ns fixed per instruction


def _windows(rmin, rmax, gamma):
    """E-range, j-windows and saturated subranges for the ramp accumulation.

    c(j) = clip(r + gamma*(j-300) + 300, 0, 600) - j.  PR_e = pm*clip(c-e,0,1)
    is nonzero where c > e (j < 300 + (rmax-e)/s, s = 1-gamma) and saturates
    to pm where c >= e+1 for every row (j <= 300 + (rmin-e-1)/s, modulo the
    bottom-clip corner at j <= (-3-r)/gamma where c = -j).
    """
    s = 1.0 - gamma
    cmin = rmin - abs(s) * 300.0 - 0.01
    cmax = rmax + abs(s) * 300.0 + 0.01
    dlo = int(np.ceil(cmin)) - 1
    dhi = int(np.floor(cmax)) + 1
    emin = dlo - 1
    jclip = -1  # largest j where any row can bottom-clip
    if gamma > 1e-9 and rmin < -300.0 * (1.0 - gamma):
        jclip = int(np.floor((-300.0 * (1.0 - gamma) - rmin) / gamma))
    wins = []
    for e in range(emin, dhi + 1):
        jlo = max(0, -e - 1)
        jhi = min(N, 601 - e)
        if abs(s) > 1e-9:
            jhi = min(jhi, int(np.ceil(300.0 + (rmax - e) / s)) + 3)
        if jlo >= jhi:
            continue
        # saturated subrange [satlo, sathi): every row has c >= e+1
        if abs(s) > 1e-9:
            sathi = min(jhi, int(np.floor(300.0 + (rmin - e - 1) / s)) - 2)
        else:
            sathi = jhi if rmin >= e + 1 else jlo
        satlo = jlo
        if jclip >= 0 and e > -jclip - 2:
            satlo = max(satlo, jclip + 1)
        sathi = max(sathi, satlo)
        wins.append((e, jlo, jhi, satlo, sathi))
    return emin, tuple(wins)


def _build(key, reps=1):
    emin, wins = key
    nc = bacc.Bacc(trn_type="TRN2")
    p_in = nc.dram_tensor("p", [PB, N], f32, kind="ExternalInput")
    # consts: [j (N) | j-300 (N) | rows (NOUTER*T*4)] packed into ONE DMA
    CW = 2 * N + NOUTER * T * 4
    consts_in = nc.dram_tensor("consts", [P, CW], f32, kind="ExternalInput")
    out = nc.dram_tensor("out", [PB, N], f32, kind="ExternalOutput")

    pt = p_in[:, :].rearrange("(o t p) m -> o t p m", t=T, p=P)
    ot = out[:, :].rearrange("(o t p) m -> o t p m", t=T, p=P)

    load = {"vector": 0.0, "gpsimd": 0.0}

    def chg(eng, kind, elems):
        load[eng] += _COST[(eng, kind)] * elems + _INSTR_OVERHEAD

    with TileContext(nc) as tc:
        with tc.tile_pool(name="const", bufs=1) as constp, \
             tc.tile_pool(name="io", bufs=3) as iop, \
             tc.tile_pool(name="acc", bufs=2) as accp, \
             tc.tile_pool(name="work", bufs=2) as workp:
            ct = constp.tile([P, CW], f32)
            nc.sync.dma_start(out=ct, in_=consts_in[:, :])
            J = ct[:, 0:N]
            Jz = ct[:, N:2 * N]
            rview = ct[:, 2 * N:].rearrange("p (o t c) -> p o t c", o=NOUTER, t=T)

            for _rep in range(reps):
              for o in range(NOUTER):
                p4 = iop.tile([P, T, N], f32, tag="p4")
                nc.gpsimd.dma_start(out=p4, in_=pt[o].rearrange("t p m -> p t m"))

                c4 = workp.tile([P, T, N], f32, tag="c4")
                H4 = accp.tile([P, T, N + 1], f32, tag="H4")
                st4 = accp.tile([P, T, N], f32, tag="st4")

                for t in range(T):
                    bpre = workp.tile([P, N], f32, tag="bpre")
                    # bpre = Jz*beta + base300
                    nc.vector.tensor_scalar(
                        out=bpre, in0=Jz, scalar1=rview[:, o, t, 0:1],
                        scalar2=rview[:, o, t, 1:2], op0=Alu.mult, op1=Alu.add)
                    chg("vector", "ts", N)
                    # bclip = clip(bpre, 0, 600)  (in place)
                    nc.vector.tensor_scalar(
                        out=bpre, in0=bpre, scalar1=0.0, scalar2=600.0,
                        op0=Alu.max, op1=Alu.min)
                    chg("vector", "ts", N)
                    # c = bclip - j
                    nc.vector.tensor_sub(c4[:, t, :], bpre, J)
                    chg("vector", "tt", N)

                nc.gpsimd.memset(H4[:, :, :], 0.0)
                chg("gpsimd", "ts", T * (N + 1))

                for (e, jlo, jhi, satlo, sathi) in wins:
                    # saturated middle: H += pm directly (DVE, reads p4)
                    if satlo < sathi:
                        nc.vector.tensor_add(
                            H4[:, :, satlo + e + 1:sathi + e + 1],
                            H4[:, :, satlo + e + 1:sathi + e + 1],
                            p4[:, :, satlo:sathi])
                        chg("vector", "tt", T * (sathi - satlo))
                    # ramp subranges: [jlo, satlo) and [sathi, jhi)
                    for (a, b) in ((jlo, satlo), (sathi, jhi)):
                        if a >= b:
                            continue
                        L = b - a
                        u4 = workp.tile([P, T, N], f32, tag="u4")
                        nc.vector.tensor_scalar(
                            out=u4[:, :, a:b], in0=c4[:, :, a:b],
                            scalar1=float(e), scalar2=0.0,
                            op0=Alu.subtract, op1=Alu.max)
                        chg("vector", "ts", T * L)
                        nc.vector.scalar_tensor_tensor(
                            out=u4[:, :, a:b], in0=u4[:, :, a:b], scalar=1.0,
                            in1=p4[:, :, a:b], op0=Alu.min, op1=Alu.mult)
                        chg("vector", "tt", T * L)
                        nc.gpsimd.tensor_add(
                            H4[:, :, a + e + 1:b + e + 1],
                            H4[:, :, a + e + 1:b + e + 1],
                            u4[:, :, a:b])
                        chg("gpsimd", "tt", T * L)

                # out[k] = H[k] - H[k+1] + pm[k - emin]
                nc.vector.tensor_sub(st4[:, :, :], H4[:, :, 0:N], H4[:, :, 1:N + 1])
                chg("vector", "tt", T * N)
                ke = N + emin
                nc.vector.tensor_add(
                    st4[:, :, 0:ke], st4[:, :, 0:ke], p4[:, :, -emin:N])
                chg("vector", "tt", T * ke)

                nc.gpsimd.dma_start(out=ot[o].rearrange("t p m -> p t m"), in_=st4)

    nc.finalize()
    return nc, load


_cache = {}


def _get_program(wins):
    if wins not in _cache:
        _cache[wins] = _build(wins)
    return _cache[wins]


def kernel(next_dist, rewards, dones, gamma):
    p = np.asarray(next_dist, dtype=np.float32)
    r = np.asarray(rewards, dtype=np.float32)
    dn = np.asarray(dones).astype(bool)
    g = float(np.asarray(gamma))

    # done rows contribute tri(clip(r+300,0,600) - k), independent of p:
    # substitute a one-hot at j=300 (c(300)=clip(r+300,0,600)-300 stays in
    # the shift window) so they ride the same pipeline.
    onehot = np.zeros(N, np.float32)
    onehot[300] = 1.0
    pm = np.where(dn[:, None], onehot[None, :], p)
    pm = np.ascontiguousarray(pm, dtype=np.float32)

    beta = np.where(dn, np.float32(0.0), np.float32(g)).astype(np.float32)
    base300 = (r + np.float32(300.0)).astype(np.float32)
    rows = np.stack([beta, base300, np.zeros_like(r), np.zeros_like(r)], axis=1)
    rows = rows.astype(np.float32)
    rows_core = []
    for c in range(NCORES):
        rc = rows[c * PB:(c + 1) * PB].reshape(NOUTER, T, P, 4)
        rc = np.ascontiguousarray(rc.transpose(2, 0, 1, 3).reshape(P, NOUTER * T * 4))
        rows_core.append(rc)

    rmin, rmax = float(r.min()), float(r.max())
    key = _windows(rmin, rmax, g)
    if key not in _cache:
        _cache[key] = _build(key)
    nc, _ = _cache[key]

    j = np.arange(N, dtype=np.float32)
    jtile = np.tile(j, (P, 1))
    jztile = np.tile((j - np.float32(300.0)).astype(np.float32), (P, 1))

    in_maps = []
    for c in range(NCORES):
        sl = slice(c * PB, (c + 1) * PB)
        consts = np.ascontiguousarray(
            np.concatenate([jtile, jztile, rows_core[c]], axis=1).astype(np.float32))
        in_maps.append({"p": pm[sl], "consts": consts})

    res = run_bass_kernel_spmd(nc, in_maps, list(range(NCORES)))
    global last_exec_time_ns, last_profile
    last_exec_time_ns = getattr(res, "exec_time_ns", None)
    last_profile = getattr(res, "profile_json", None)
    outs = [res.results[c]["out"] for c in range(NCORES)]
    return np.concatenate(outs, axis=0).astype(np.float32)


last_exec_time_ns = None
last_profile = None
